# revision 23
# baseline (speedup 1.0000x reference)
"""Trainium2 Bass kernel for nn_KnowledgeAttention.

Math (per batch example b):
    sim[k]  = cos_sim(pooled[b], kg_key[b,k])                      # [K]
    q       = (hs @ Wq.T + bq) * HD**-0.5     -> heads [T,H,HD]
    k       = kg_value @ Wk.T + bk            -> heads [K,H,HD]
    v       = kg_value @ Wv.T + bv            -> heads [K,H,HD]
    S[h,t,k]= q_h[t]·k_h[k] + beta[h]*sim[k]
    P       = softmax_k(S);  O[t,h] = sum_k P v
    out     = O @ Wo.T + bo
Since exp(s + beta*sim) = exp(s) * f with f[k,h] = exp(beta[h]*sim[k]),
the per-head bias is folded into f-scaled V columns and f-valued
denominator weights, so the exp itself is bias-free and can process the
even and odd head of a pair in ONE activation instruction over a 2-bank
PSUM tile.

Sharding: pure data-parallel over batch — 8 examples on 8 cores, weights
replicated, no collectives.

Per-core layout (all matmul contractions on the partition dim):
    hs.T/kg_value.T via PE transposes in bf16; scores computed transposed
    S.T[k,t]; even/odd heads of a pair run as row-tiled concurrent
    matmuls into the two halves of a [128,1024] PSUM tile; one bias-free
    Exp covers both; AV and the (64-row-replicated) denominator matmuls
    run col-tiled concurrently; normalization is one reciprocal + one
    multiply per pair.
"""

import sys

import numpy as np

# ---------------------------------------------------------------- constants
BS = 8
T = 2048
D = 768
H = 12
HD = 64
K = 512
SCALE = HD ** -0.5
EPS = 1e-8
DC = D // 128   # 6 contraction/partition chunks of 128 over D
KC = K // 128   # 4 chunks over K
TW = 512        # t window for moving operand
NTW = T // TW   # 4
NPAIR = H // 2  # 6 head pairs

FUSED_EXP = True   # one Exp over [128,1024] spanning 2 PSUM banks

TRACE = False
LAST_EXEC_NS = None

_CACHE = {}


def _ensure_path():
    try:
        import concourse  # noqa: F401
    except ImportError:
        for p in ("/opt/trn_rl_repo", "/root/.axon_site/_ro/trn_rl_repo"):
            if p not in sys.path:
                sys.path.insert(0, p)


def _build_program():
    _ensure_path()
    import concourse.bass as bass
    import concourse.mybir as mybir
    import concourse.tile as tile
    from concourse import bacc
    from concourse.masks import make_identity
    from contextlib import ExitStack

    F32 = mybir.dt.float32
    BF16 = mybir.dt.bfloat16
    Alu = mybir.AluOpType
    Act = mybir.ActivationFunctionType

    nc = bacc.Bacc("TRN2", target_bir_lowering=False, debug=False, num_devices=BS)

    hs_d = nc.dram_tensor("hs", [T, D], BF16, kind="ExternalInput").ap()
    kgk_d = nc.dram_tensor("kgk", [K, D], F32, kind="ExternalInput").ap()
    kgv_d = nc.dram_tensor("kgv", [K, D], BF16, kind="ExternalInput").ap()
    pl_d = nc.dram_tensor("pooled", [1, D], F32, kind="ExternalInput").ap()
    wqt_d = nc.dram_tensor("wqt", [D, D], BF16, kind="ExternalInput").ap()
    wkt_d = nc.dram_tensor("wkt", [D, D], BF16, kind="ExternalInput").ap()
    wvt_d = nc.dram_tensor("wvt", [D, D], BF16, kind="ExternalInput").ap()
    wot_d = nc.dram_tensor("wot", [D, D], BF16, kind="ExternalInput").ap()
    bq_d = nc.dram_tensor("bq", [128, DC], F32, kind="ExternalInput").ap()
    bk_d = nc.dram_tensor("bk", [128, DC], F32, kind="ExternalInput").ap()
    bo_d = nc.dram_tensor("bo", [1, D], F32, kind="ExternalInput").ap()
    beta_d = nc.dram_tensor("beta", [1, H], F32, kind="ExternalInput").ap()
    out_d = nc.dram_tensor("out", [T, D], F32, kind="ExternalOutput").ap()

    with tile.TileContext(nc) as tc, ExitStack() as ctx:
        const = ctx.enter_context(tc.tile_pool(name="const", bufs=1))
        inp = ctx.enter_context(tc.tile_pool(name="inp", bufs=8))
        wpool = ctx.enter_context(tc.tile_pool(name="w", bufs=24))
        big = ctx.enter_context(tc.tile_pool(name="big", bufs=12))
        hstw_p = ctx.enter_context(tc.tile_pool(name="hstw", bufs=12))
        kt_p = ctx.enter_context(tc.tile_pool(name="ktp", bufs=6))
        v_p = ctx.enter_context(tc.tile_pool(name="vp", bufs=4))
        kgt_p = ctx.enter_context(tc.tile_pool(name="kgtp", bufs=6))
        e_p = ctx.enter_context(tc.tile_pool(name="ep", bufs=10))
        r_p = ctx.enter_context(tc.tile_pool(name="rp", bufs=2))
        fin_p = ctx.enter_context(tc.tile_pool(name="finp", bufs=2))
        sm_p = ctx.enter_context(tc.tile_pool(name="smp", bufs=4))
        ps = ctx.enter_context(tc.tile_pool(name="ps", bufs=1, space="PSUM"))

        # ---------------- phase 0: constants + cosine-sim factors ----------------
        # HAM warm-up: keep the PE busy through the whole initial DMA wait so
        # the clock gate opens (K=8/8) and stays open when real work arrives.
        warm = const.tile([128, 512], BF16, tag="warm")
        nc.vector.memset(warm[:], 0.0)
        for i in range(34):
            pwarm = ps.tile([128, 1024], F32, tag="s", bufs=2, name="pwarm")
            nc.tensor.matmul(pwarm[:, 0:512], warm[:, 0:128], warm[:],
                             start=True, stop=True)

        ident = const.tile([128, 128], BF16, tag="ident")
        make_identity(nc, ident[:])
        ones64 = const.tile([128, 64], BF16, tag="ones64")
        nc.vector.memset(ones64[:], 1.0)
        # kg_value + hs window 0 load first: transposes are the critical path
        kv_tiles = []
        for c in range(KC):
            kv = inp.tile([128, D], BF16, tag="inp", name="kv")
            nc.sync.dma_start(kv[:], kgv_d[c * 128:(c + 1) * 128, :])
            kv_tiles.append(kv)

        hv0 = []
        for tsub in range(TW // 128):
            hv = inp.tile([128, D], BF16, tag="hv", bufs=10, name="hv")
            nc.sync.dma_start(hv[:], hs_d[tsub * 128:tsub * 128 + 128, :])
            hv0.append(hv)

        pl = const.tile([1, D], F32, tag="pl")
        nc.sync.dma_start(pl[:], pl_d)
        bt = const.tile([1, H], F32, tag="bt")
        nc.sync.dma_start(bt[:], beta_d)
        bo_row = const.tile([1, D], F32, tag="bo_row")
        nc.sync.dma_start(bo_row[:], bo_d)
        bq_sb = const.tile([128, DC], F32, tag="bq_sb")
        nc.sync.dma_start(bq_sb[:], bq_d)
        bk_sb = const.tile([128, DC], F32, tag="bk_sb")
        nc.sync.dma_start(bk_sb[:], bk_d)

        wk_sb = []
        wq_sb = []
        for c in range(DC):
            wk = wpool.tile([128, D], BF16, tag="w")
            nc.sync.dma_start(wk[:], wkt_d[c * 128:(c + 1) * 128, :])
            wk_sb.append(wk)
        for c in range(DC):
            wq = wpool.tile([128, D], BF16, tag="w")
            nc.sync.dma_start(wq[:], wqt_d[c * 128:(c + 1) * 128, :])
            wq_sb.append(wq)

        bo_bc = const.tile([128, D], F32, tag="bo_bc")
        nc.gpsimd.partition_broadcast(bo_bc[:], bo_row[:], channels=128)
        beta_bc = const.tile([128, H], F32, tag="beta_bc")
        nc.gpsimd.partition_broadcast(beta_bc[:], bt[:], channels=128)
        pl_bc = const.tile([128, D], F32, tag="pl_bc")
        nc.gpsimd.partition_broadcast(pl_bc[:], pl[:], channels=128)

        # pooled 1/||.|| as a per-partition vector (computed on the broadcast)
        pl_sq = inp.tile([128, D], F32, tag="inp")
        pnorm = sm_p.tile([128, 1], F32, tag="pnorm")
        nc.scalar.activation(pl_sq[:], pl_bc[:], Act.Square, accum_out=pnorm[:])
        nc.scalar.activation(pnorm[:], pnorm[:], Act.Sqrt)
        nc.vector.tensor_scalar_max(pnorm[:], pnorm[:], EPS)
        rp_vec = const.tile([128, 1], F32, tag="rp_vec")
        nc.vector.reciprocal(rp_vec[:], pnorm[:])

        # bias_all[k_part, kc*H + h] = beta[h] * sim[k]; f_all = exp(bias_all)
        bias_all = const.tile([128, KC * H], F32, tag="bias_all")
        for c in range(KC):
            kk = inp.tile([128, D], F32, tag="inp")
            nc.sync.dma_start(kk[:], kgk_d[c * 128:(c + 1) * 128, :])
            sq = inp.tile([128, D], F32, tag="inp")
            nrm = sm_p.tile([128, 1], F32, tag="nrm")
            nc.scalar.activation(sq[:], kk[:], Act.Square, accum_out=nrm[:])
            nc.scalar.activation(nrm[:], nrm[:], Act.Sqrt)
            nc.vector.tensor_scalar_max(nrm[:], nrm[:], EPS)
            rn = sm_p.tile([128, 1], F32, tag="rn")
            nc.vector.reciprocal(rn[:], nrm[:])
            sq2 = inp.tile([128, D], F32, tag="inp")
            dot = sm_p.tile([128, 1], F32, tag="dot")
            nc.vector.scalar_tensor_tensor(
                out=sq2[:], in0=kk[:], scalar=1.0, in1=pl_bc[:],
                op0=Alu.mult, op1=Alu.mult, accum_out=dot[:])
            nc.vector.tensor_mul(dot[:], dot[:], rn[:])
            nc.vector.tensor_mul(dot[:], dot[:], rp_vec[:])
            nc.vector.tensor_scalar_mul(
                bias_all[:, c * H:(c + 1) * H], beta_bc[:], dot[:])
        f_all = const.tile([128, KC * H], F32, tag="f_all")
        nc.scalar.activation(f_all[:], bias_all[:], Act.Exp)

        # ---------------- phase 1a: kg_value.T + hs-w0.T ----------------
        kgt = [kgt_p.tile([128, K], BF16, tag="kgt", name="kgt") for _ in range(DC)]
        for dchunk in range(DC):
            pt = ps.tile([128, K], BF16, tag="mm", bufs=2, name="ptr")
            for c in range(KC):
                nc.tensor.transpose(
                    pt[:, c * 128:(c + 1) * 128],
                    kv_tiles[c][:, dchunk * 128:(dchunk + 1) * 128], ident[:])
            nc.vector.tensor_copy(kgt[dchunk][:], pt[:])

        qt = [big.tile([128, T], BF16, tag="big", name="qt") for _ in range(DC)]
        kt = [kt_p.tile([128, K], BF16, tag="kt", name="kt") for _ in range(DC)]
        hstw_w = [[None] * DC for _ in range(NTW)]

        def load_hs_window(w):
            hv_tiles = []
            for tsub in range(TW // 128):
                hv = inp.tile([128, D], BF16, tag="hv", bufs=10, name="hv")
                t0 = w * TW + tsub * 128
                nc.sync.dma_start(hv[:], hs_d[t0:t0 + 128, :])
                hv_tiles.append(hv)
            return hv_tiles

        def transpose_window(w, hv_tiles, cs):
            for c in cs:
                pt = ps.tile([128, TW], BF16, tag="mm", bufs=2, name="ptr")
                for tsub in range(TW // 128):
                    nc.tensor.transpose(
                        pt[:, tsub * 128:(tsub + 1) * 128],
                        hv_tiles[tsub][:, c * 128:(c + 1) * 128], ident[:])
                hstw_w[w][c] = hstw_p.tile(
                    [128, TW], BF16, tag="hstw", name="hstw")
                nc.vector.tensor_copy(hstw_w[w][c][:], pt[:])

        def qproj_chain(w, m):
            pq = ps.tile([128, TW], F32, tag="mm", bufs=2, name="pq")
            for c in range(DC):
                nc.tensor.matmul(
                    pq[:], wq_sb[c][:, m * 128:(m + 1) * 128],
                    hstw_w[w][c][:],
                    start=(c == 0), stop=(c == DC - 1))
            nc.vector.tensor_scalar_add(
                qt[m][:, w * TW:(w + 1) * TW], pq[:], bq_sb[:, m:m + 1])

        def kproj_chain(m):
            pk = ps.tile([128, K], F32, tag="mm", bufs=2, name="pk")
            for c in range(DC):
                nc.tensor.matmul(
                    pk[:], wk_sb[c][:, m * 128:(m + 1) * 128], kgt[c][:],
                    start=(c == 0), stop=(c == DC - 1))
            nc.vector.tensor_scalar_add(kt[m][:], pk[:], bk_sb[:, m:m + 1])

        transpose_window(0, hv0, list(range(DC)))

        # weights + inputs needed later, in the order they will be consumed
        wv_sb = []
        wo_sb = []
        for c in range(DC):
            wv = wpool.tile([128, D], BF16, tag="w")
            nc.sync.dma_start(wv[:], wvt_d[c * 128:(c + 1) * 128, :])
            wv_sb.append(wv)
        hv_next = load_hs_window(1)
        for c in range(DC):
            wo = wpool.tile([128, D], BF16, tag="w")
            nc.sync.dma_start(wo[:], wot_d[c * 128:(c + 1) * 128, :])
            wo_sb.append(wo)

        # v_sb[kc][:, h*64:(h+1)*64] = f_all[:, kc*H+h] * (kg_value @ Wv.T)_h
        v_sb = [v_p.tile([128, D], BF16, tag="v", name="vsb")
                for _ in range(KC)]

        def vproj_half(n):
            for kc in range(KC):
                pv = ps.tile([128, 384], F32, tag="mm", bufs=2, name="pv")
                for c in range(DC):
                    nc.tensor.matmul(
                        pv[:], kgt[c][:, kc * 128:(kc + 1) * 128],
                        wv_sb[c][:, n * 384:(n + 1) * 384],
                        start=(c == 0), stop=(c == DC - 1))
                for hh in range(6):
                    h = n * 6 + hh
                    nc.vector.tensor_scalar_mul(
                        v_sb[kc][:, h * 64:(h + 1) * 64],
                        pv[:, hh * 64:(hh + 1) * 64],
                        f_all[:, kc * H + h:kc * H + h + 1])

        def outproj_chunk(tc16):
            # c-outer with n0/n1 interleaved: consecutive matmuls share the
            # same stationary ot[c] tile
            fin = fin_p.tile([128, D], F32, tag="fin")
            pf0 = ps.tile([128, 384], F32, tag="mm", bufs=2, name="pf0")
            pf1 = ps.tile([128, 384], F32, tag="mm", bufs=2, name="pf1")
            for c in range(DC):
                lhs = ot[c][:, tc16 * 128:(tc16 + 1) * 128]
                nc.tensor.matmul(pf0[:], lhs, wo_sb[c][:, 0:384],
                                 start=(c == 0), stop=(c == DC - 1))
                nc.tensor.matmul(pf1[:], lhs, wo_sb[c][:, 384:768],
                                 start=(c == 0), stop=(c == DC - 1))
            nc.vector.tensor_add(fin[:, 0:384], pf0[:], bo_bc[:, 0:384])
            nc.sync.dma_start(out_d[tc16 * 128:(tc16 + 1) * 128, 0:384],
                              fin[:, 0:384])
            nc.vector.tensor_add(fin[:, 384:768], pf1[:], bo_bc[:, 384:768])
            nc.sync.dma_start(out_d[tc16 * 128:(tc16 + 1) * 128, 384:768],
                              fin[:, 384:768])

        # denominator weights: dwt[:, (kc*H+h)*64 : +64] = f_all[:, kc*H+h] (x64)
        dwt = const.tile([128, KC * H * 64], BF16, tag="dwt")

        def dwt_build(hs_range):
            for h in hs_range:
                for kc in range(KC):
                    col = kc * H + h
                    nc.vector.tensor_scalar_mul(
                        dwt[:, col * 64:(col + 1) * 64], ones64[:],
                        f_all[:, col:col + 1])

        def dw(kc, h):
            col = kc * H + h
            return dwt[:, col * 64:(col + 1) * 64]

        # ------- phase 2+3: attention + final projection per t-window -------
        ot = [big.tile([128, T], BF16, tag="big", name="ot") for _ in range(NPAIR)]
        for w in range(NTW):
            tw = slice(w * TW, (w + 1) * TW)
            for j in range(NPAIR):
                he, ho = 2 * j, 2 * j + 1
                if w == 0:
                    kproj_chain(j)
                    qproj_chain(0, j)
                # --- scores: even/odd row-tiled into the 2 banks of sc2
                e_tiles = []
                for kc in range(KC):
                    sc2 = ps.tile([128, 2 * TW], F32, tag="s", bufs=2,
                                  name="sc2")
                    nc.tensor.matmul(
                        sc2[:, 0:TW], kt[j][0:64, kc * 128:(kc + 1) * 128],
                        qt[j][0:64, tw], start=True, stop=True,
                        tile_position=(0, 0))
                    nc.tensor.matmul(
                        sc2[:, TW:2 * TW], kt[j][64:128, kc * 128:(kc + 1) * 128],
                        qt[j][64:128, tw], start=True, stop=True,
                        tile_position=(64, 0))
                    ee = e_p.tile([128, 2 * TW], BF16, tag="e", name="ee")
                    if FUSED_EXP:
                        nc.scalar.activation(ee[:], sc2[:], Act.Exp)
                    else:
                        nc.scalar.activation(ee[:, 0:TW], sc2[:, 0:TW], Act.Exp)
                        nc.scalar.activation(
                            ee[:, TW:2 * TW], sc2[:, TW:2 * TW], Act.Exp)
                    e_tiles.append(ee)

                if w == 0 and j == 0:
                    vproj_half(0)
                    dwt_build(range(0, 6))
                elif w == 0 and j == 1:
                    vproj_half(1)
                    dwt_build(range(6, H))

                # --- AV + denominator, col-tiled concurrent chains
                po = ps.tile([128, TW], F32, tag="o", bufs=1, name="po")
                pd = ps.tile([128, TW], F32, tag="d", bufs=1, name="pd")
                # phase A: AV even head (cols 0:64) || denom odd head (cols 64:128)
                for kc in range(KC):
                    nc.tensor.matmul(
                        po[0:64, :], v_sb[kc][:, he * HD:(he + 1) * HD],
                        e_tiles[kc][:, 0:TW],
                        start=(kc == 0), stop=(kc == KC - 1))
                    nc.tensor.matmul(
                        pd[64:128, :], dw(kc, ho),
                        e_tiles[kc][:, TW:2 * TW],
                        start=(kc == 0), stop=(kc == KC - 1))
                # phase B: AV odd head (cols 64:128) || denom even head (cols 0:64)
                for kc in range(KC):
                    nc.tensor.matmul(
                        po[64:128, :], v_sb[kc][:, ho * HD:(ho + 1) * HD],
                        e_tiles[kc][:, TW:2 * TW],
                        start=(kc == 0), stop=(kc == KC - 1))
                    nc.tensor.matmul(
                        pd[0:64, :], dw(kc, he),
                        e_tiles[kc][:, 0:TW],
                        start=(kc == 0), stop=(kc == KC - 1))

                rall = r_p.tile([128, TW], F32, tag="rall", name="rall")
                nc.vector.reciprocal_approx_fast(rall[:], pd[:])
                nc.vector.tensor_mul(ot[j][:, tw], po[:], rall[:])

                # software pipeline: next window's transposes + q-proj,
                # spread across this window's pairs
                if w < NTW - 1:
                    if j == 1:
                        transpose_window(w + 1, hv_next, [0, 1, 2])
                    elif j == 3:
                        transpose_window(w + 1, hv_next, [3, 4, 5])
                    elif j == 4:
                        for m in range(3):
                            qproj_chain(w + 1, m)
                        if w < NTW - 2:
                            hv_next = load_hs_window(w + 2)
                    elif j == 5:
                        for m in range(3, DC):
                            qproj_chain(w + 1, m)

            # --- final projection for this window
            for tsub in range(TW // 128):
                outproj_chunk(w * (TW // 128) + tsub)

    nc.compile()
    return nc


def _get_program():
    if "nc" not in _CACHE:
        _CACHE["nc"] = _build_program()
    return _CACHE["nc"]


def _host_prep(inputs):
    import ml_dtypes
    bf16 = ml_dtypes.bfloat16

    f32 = lambda x: np.ascontiguousarray(np.asarray(x, dtype=np.float32))
    Wq, Wk, Wv, Wo = (f32(inputs[k]) for k in ("Wq", "Wk", "Wv", "Wo"))
    bq, bk, bv, bo = (f32(inputs[k]) for k in ("bq", "bk", "bv", "bo"))
    beta = f32(inputs["beta"])

    shared = {
        "wqt": np.ascontiguousarray((Wq.T * SCALE).astype(bf16)),
        "wkt": np.ascontiguousarray(Wk.T.astype(bf16)),
        "wvt": np.ascontiguousarray(Wv.T.astype(bf16)),
        "wot": np.ascontiguousarray(Wo.T.astype(bf16)),
        "bq": np.ascontiguousarray((bq * SCALE).reshape(DC, 128).T),
        "bk": np.ascontiguousarray(bk.reshape(DC, 128).T),
        # bv folded through Wo (sum_k softmax == 1), bo absorbed:
        "bo": np.ascontiguousarray((bo + bv @ Wo.T).reshape(1, D)),
        "beta": np.ascontiguousarray(beta.reshape(1, H)),
    }

    hs = np.asarray(inputs["hidden_states"])
    kgk = f32(inputs["kg_key"])
    kgv = np.asarray(inputs["kg_value"])
    pooled = f32(inputs["pooled_hidden_states"])
    hs_bf = hs.astype(bf16)
    kgv_bf = kgv.astype(bf16)

    in_maps = []
    for b in range(BS):
        m = dict(shared)
        m["hs"] = np.ascontiguousarray(hs_bf[b])
        m["kgk"] = np.ascontiguousarray(kgk[b])
        m["kgv"] = np.ascontiguousarray(kgv_bf[b])
        m["pooled"] = np.ascontiguousarray(pooled[b].reshape(1, D))
        in_maps.append(m)
    return in_maps




def _install_ntff_hook():
    """Register the axon NTFF profile hook so trace=True yields exec_time_ns.

    Only used from our own test harness (TRACE=True); the default kernel()
    path never calls this.
    """
    try:
        from antenv.axon_hooks import get_axon_ntff_profile_hook  # noqa: F401
        return
    except ImportError:
        pass
    import contextlib
    import ctypes
    import types

    so_path = "/opt/axon/libaxon_pjrt.so"
    try:
        lib = ctypes.CDLL(so_path)
    except OSError:
        return
    if not hasattr(lib, "axon_start_nrt_profile"):
        return
    lib.axon_start_nrt_profile.argtypes = [
        ctypes.POINTER(ctypes.c_int64), ctypes.c_size_t]
    lib.axon_start_nrt_profile.restype = ctypes.c_int64
    lib.axon_stop_nrt_profile.argtypes = [ctypes.c_char_p]
    lib.axon_stop_nrt_profile.restype = ctypes.c_int64

    @contextlib.contextmanager
    def _hook(output_dir, device_ids):
        import jax
        jax.devices()
        if device_ids:
            ids = (ctypes.c_int64 * len(device_ids))(*device_ids)
            rc = lib.axon_start_nrt_profile(ids, len(device_ids))
        else:
            rc = lib.axon_start_nrt_profile(None, 0)
        if rc != 0:
            raise RuntimeError(f"axon_start_nrt_profile rc={rc}")
        try:
            yield
        finally:
            n = lib.axon_stop_nrt_profile(str(output_dir).encode())
            print(f"profile: {n} file(s) written to {output_dir}",
                  file=sys.stderr)

    mod = types.ModuleType("antenv.axon_hooks")
    mod.get_axon_ntff_profile_hook = lambda: _hook
    mod.set_axon_ntff_profile_hook = lambda h: None
    sys.modules["antenv.axon_hooks"] = mod


def kernel(**inputs):
    global LAST_EXEC_NS
    _ensure_path()
    from concourse import bass_utils

    if TRACE:
        _install_ntff_hook()
    nc = _get_program()
    in_maps = _host_prep(inputs)
    res = bass_utils.run_bass_kernel_spmd(
        nc, in_maps, core_ids=list(range(BS)), trace=TRACE)
    LAST_EXEC_NS = res.exec_time_ns
    out = np.stack([res.results[b]["out"] for b in range(BS)], axis=0)
    return out.astype(np.float32)


# revision 26
# speedup vs baseline: 1.0051x; 1.0051x over previous
"""Trainium2 Bass kernel for nn_KnowledgeAttention.

Math (per batch example b):
    sim[k]  = cos_sim(pooled[b], kg_key[b,k])                      # [K]
    q       = (hs @ Wq.T + bq) * HD**-0.5     -> heads [T,H,HD]
    k       = kg_value @ Wk.T + bk            -> heads [K,H,HD]
    v       = kg_value @ Wv.T + bv            -> heads [K,H,HD]
    S[h,t,k]= q_h[t]·k_h[k] + beta[h]*sim[k]
    P       = softmax_k(S);  O[t,h] = sum_k P v
    out     = O @ Wo.T + bo
Since exp(s + beta*sim) = exp(s) * f with f[k,h] = exp(beta[h]*sim[k]),
the per-head bias is folded into f-scaled V columns and f-valued
denominator weights, so the exp itself is bias-free and can process the
even and odd head of a pair in ONE activation instruction over a 2-bank
PSUM tile.

Sharding: pure data-parallel over batch — 8 examples on 8 cores, weights
replicated, no collectives.

Per-core layout (all matmul contractions on the partition dim):
    hs.T/kg_value.T via PE transposes in bf16; scores computed transposed
    S.T[k,t]; even/odd heads of a pair run as row-tiled concurrent
    matmuls into the two halves of a [128,1024] PSUM tile; one bias-free
    Exp covers both; AV and the (64-row-replicated) denominator matmuls
    run col-tiled concurrently; normalization is one reciprocal + one
    multiply per pair.
"""

import sys

import numpy as np

# ---------------------------------------------------------------- constants
BS = 8
T = 2048
D = 768
H = 12
HD = 64
K = 512
SCALE = HD ** -0.5
EPS = 1e-8
DC = D // 128   # 6 contraction/partition chunks of 128 over D
KC = K // 128   # 4 chunks over K
TW = 512        # t window for moving operand
NTW = T // TW   # 4
NPAIR = H // 2  # 6 head pairs

FUSED_EXP = True   # one Exp over [128,1024] spanning 2 PSUM banks

TRACE = False
LAST_EXEC_NS = None

_CACHE = {}


def _ensure_path():
    try:
        import concourse  # noqa: F401
    except ImportError:
        for p in ("/opt/trn_rl_repo", "/root/.axon_site/_ro/trn_rl_repo"):
            if p not in sys.path:
                sys.path.insert(0, p)


def _build_program():
    _ensure_path()
    import concourse.bass as bass
    import concourse.mybir as mybir
    import concourse.tile as tile
    from concourse import bacc
    from concourse.masks import make_identity
    from contextlib import ExitStack

    F32 = mybir.dt.float32
    BF16 = mybir.dt.bfloat16
    Alu = mybir.AluOpType
    Act = mybir.ActivationFunctionType

    nc = bacc.Bacc("TRN2", target_bir_lowering=False, debug=False, num_devices=BS)

    hs_d = nc.dram_tensor("hs", [T, D], BF16, kind="ExternalInput").ap()
    kgk_d = nc.dram_tensor("kgk", [K, D], F32, kind="ExternalInput").ap()
    kgv_d = nc.dram_tensor("kgv", [K, D], BF16, kind="ExternalInput").ap()
    pl_d = nc.dram_tensor("pooled", [1, D], F32, kind="ExternalInput").ap()
    wqt_d = nc.dram_tensor("wqt", [D, D], BF16, kind="ExternalInput").ap()
    wkt_d = nc.dram_tensor("wkt", [D, D], BF16, kind="ExternalInput").ap()
    wvt_d = nc.dram_tensor("wvt", [D, D], BF16, kind="ExternalInput").ap()
    wot_d = nc.dram_tensor("wot", [D, D], BF16, kind="ExternalInput").ap()
    bq_d = nc.dram_tensor("bq", [128, DC], F32, kind="ExternalInput").ap()
    bk_d = nc.dram_tensor("bk", [128, DC], F32, kind="ExternalInput").ap()
    bo_d = nc.dram_tensor("bo", [1, D], F32, kind="ExternalInput").ap()
    beta_d = nc.dram_tensor("beta", [1, H], F32, kind="ExternalInput").ap()
    out_d = nc.dram_tensor("out", [T, D], F32, kind="ExternalOutput").ap()

    with tile.TileContext(nc) as tc, ExitStack() as ctx:
        const = ctx.enter_context(tc.tile_pool(name="const", bufs=1))
        inp = ctx.enter_context(tc.tile_pool(name="inp", bufs=8))
        wpool = ctx.enter_context(tc.tile_pool(name="w", bufs=24))
        big = ctx.enter_context(tc.tile_pool(name="big", bufs=12))
        hstw_p = ctx.enter_context(tc.tile_pool(name="hstw", bufs=12))
        kt_p = ctx.enter_context(tc.tile_pool(name="ktp", bufs=6))
        v_p = ctx.enter_context(tc.tile_pool(name="vp", bufs=4))
        kgt_p = ctx.enter_context(tc.tile_pool(name="kgtp", bufs=6))
        e_p = ctx.enter_context(tc.tile_pool(name="ep", bufs=10))
        r_p = ctx.enter_context(tc.tile_pool(name="rp", bufs=2))
        fin_p = ctx.enter_context(tc.tile_pool(name="finp", bufs=2))
        sm_p = ctx.enter_context(tc.tile_pool(name="smp", bufs=4))
        ps = ctx.enter_context(tc.tile_pool(name="ps", bufs=1, space="PSUM"))

        # ---------------- phase 0: constants + cosine-sim factors ----------------
        # HAM warm-up: keep the PE busy through the whole initial DMA wait so
        # the clock gate opens (K=8/8) and stays open when real work arrives.
        warm = const.tile([128, 512], BF16, tag="warm")
        nc.vector.memset(warm[:], 0.0)
        for i in range(34):
            pwarm = ps.tile([128, 1024], F32, tag="s", bufs=2, name="pwarm")
            nc.tensor.matmul(pwarm[:, 0:512], warm[:, 0:128], warm[:],
                             start=True, stop=True)

        ident = const.tile([128, 128], BF16, tag="ident")
        make_identity(nc, ident[:])
        ones64 = const.tile([128, 64], BF16, tag="ones64")
        nc.vector.memset(ones64[:], 1.0)
        # kg_value + hs window 0 load first: transposes are the critical path
        kv_tiles = []
        for c in range(KC):
            kv = inp.tile([128, D], BF16, tag="inp", name="kv")
            nc.sync.dma_start(kv[:], kgv_d[c * 128:(c + 1) * 128, :])
            kv_tiles.append(kv)

        wk_sb = []
        for c in range(DC):
            wk = wpool.tile([128, D], BF16, tag="w")
            nc.sync.dma_start(wk[:], wkt_d[c * 128:(c + 1) * 128, :])
            wk_sb.append(wk)

        hv0 = []
        for tsub in range(TW // 128):
            hv = inp.tile([128, D], BF16, tag="hv", bufs=10, name="hv")
            nc.sync.dma_start(hv[:], hs_d[tsub * 128:tsub * 128 + 128, :])
            hv0.append(hv)

        wq_sb = []
        for c in range(DC):
            wq = wpool.tile([128, D], BF16, tag="w")
            nc.sync.dma_start(wq[:], wqt_d[c * 128:(c + 1) * 128, :])
            wq_sb.append(wq)

        pl = const.tile([1, D], F32, tag="pl")
        nc.sync.dma_start(pl[:], pl_d)
        bt = const.tile([1, H], F32, tag="bt")
        nc.sync.dma_start(bt[:], beta_d)
        bo_row = const.tile([1, D], F32, tag="bo_row")
        nc.sync.dma_start(bo_row[:], bo_d)
        bq_sb = const.tile([128, DC], F32, tag="bq_sb")
        nc.sync.dma_start(bq_sb[:], bq_d)
        bk_sb = const.tile([128, DC], F32, tag="bk_sb")
        nc.sync.dma_start(bk_sb[:], bk_d)

        bo_bc = const.tile([128, D], F32, tag="bo_bc")
        nc.gpsimd.partition_broadcast(bo_bc[:], bo_row[:], channels=128)
        beta_bc = const.tile([128, H], F32, tag="beta_bc")
        nc.gpsimd.partition_broadcast(beta_bc[:], bt[:], channels=128)
        pl_bc = const.tile([128, D], F32, tag="pl_bc")
        nc.gpsimd.partition_broadcast(pl_bc[:], pl[:], channels=128)

        # pooled 1/||.|| as a per-partition vector (computed on the broadcast)
        pl_sq = inp.tile([128, D], F32, tag="inp")
        pnorm = sm_p.tile([128, 1], F32, tag="pnorm")
        nc.scalar.activation(pl_sq[:], pl_bc[:], Act.Square, accum_out=pnorm[:])
        nc.scalar.activation(pnorm[:], pnorm[:], Act.Sqrt)
        nc.vector.tensor_scalar_max(pnorm[:], pnorm[:], EPS)
        rp_vec = const.tile([128, 1], F32, tag="rp_vec")
        nc.vector.reciprocal(rp_vec[:], pnorm[:])

        # bias_all[k_part, kc*H + h] = beta[h] * sim[k]; f_all = exp(bias_all)
        bias_all = const.tile([128, KC * H], F32, tag="bias_all")
        for c in range(KC):
            kk = inp.tile([128, D], F32, tag="inp")
            nc.sync.dma_start(kk[:], kgk_d[c * 128:(c + 1) * 128, :])
            sq = inp.tile([128, D], F32, tag="inp")
            nrm = sm_p.tile([128, 1], F32, tag="nrm")
            nc.scalar.activation(sq[:], kk[:], Act.Square, accum_out=nrm[:])
            nc.scalar.activation(nrm[:], nrm[:], Act.Sqrt)
            nc.vector.tensor_scalar_max(nrm[:], nrm[:], EPS)
            rn = sm_p.tile([128, 1], F32, tag="rn")
            nc.vector.reciprocal(rn[:], nrm[:])
            sq2 = inp.tile([128, D], F32, tag="inp")
            dot = sm_p.tile([128, 1], F32, tag="dot")
            nc.vector.scalar_tensor_tensor(
                out=sq2[:], in0=kk[:], scalar=1.0, in1=pl_bc[:],
                op0=Alu.mult, op1=Alu.mult, accum_out=dot[:])
            nc.vector.tensor_mul(dot[:], dot[:], rn[:])
            nc.vector.tensor_mul(dot[:], dot[:], rp_vec[:])
            nc.vector.tensor_scalar_mul(
                bias_all[:, c * H:(c + 1) * H], beta_bc[:], dot[:])
        f_all = const.tile([128, KC * H], F32, tag="f_all")
        nc.scalar.activation(f_all[:], bias_all[:], Act.Exp)

        # ---------------- phase 1a: kg_value.T + hs-w0.T ----------------
        kgt = [kgt_p.tile([128, K], BF16, tag="kgt", name="kgt") for _ in range(DC)]
        for dchunk in range(DC):
            pt = ps.tile([128, K], BF16, tag="mm", bufs=2, name="ptr")
            for c in range(KC):
                nc.tensor.transpose(
                    pt[:, c * 128:(c + 1) * 128],
                    kv_tiles[c][:, dchunk * 128:(dchunk + 1) * 128], ident[:])
            nc.vector.tensor_copy(kgt[dchunk][:], pt[:])

        qt = [big.tile([128, T], BF16, tag="big", name="qt") for _ in range(DC)]
        kt = [kt_p.tile([128, K], BF16, tag="kt", name="kt") for _ in range(DC)]
        hstw_w = [[None] * DC for _ in range(NTW)]

        def load_hs_window(w):
            hv_tiles = []
            for tsub in range(TW // 128):
                hv = inp.tile([128, D], BF16, tag="hv", bufs=10, name="hv")
                t0 = w * TW + tsub * 128
                nc.sync.dma_start(hv[:], hs_d[t0:t0 + 128, :])
                hv_tiles.append(hv)
            return hv_tiles

        def transpose_window(w, hv_tiles, cs):
            for c in cs:
                pt = ps.tile([128, TW], BF16, tag="mm", bufs=2, name="ptr")
                for tsub in range(TW // 128):
                    nc.tensor.transpose(
                        pt[:, tsub * 128:(tsub + 1) * 128],
                        hv_tiles[tsub][:, c * 128:(c + 1) * 128], ident[:])
                hstw_w[w][c] = hstw_p.tile(
                    [128, TW], BF16, tag="hstw", name="hstw")
                nc.vector.tensor_copy(hstw_w[w][c][:], pt[:])

        def qproj_chain(w, m):
            pq = ps.tile([128, TW], F32, tag="mm", bufs=2, name="pq")
            for c in range(DC):
                nc.tensor.matmul(
                    pq[:], wq_sb[c][:, m * 128:(m + 1) * 128],
                    hstw_w[w][c][:],
                    start=(c == 0), stop=(c == DC - 1))
            nc.vector.tensor_scalar_add(
                qt[m][:, w * TW:(w + 1) * TW], pq[:], bq_sb[:, m:m + 1])

        def kproj_chain(m):
            pk = ps.tile([128, K], F32, tag="mm", bufs=2, name="pk")
            for c in range(DC):
                nc.tensor.matmul(
                    pk[:], wk_sb[c][:, m * 128:(m + 1) * 128], kgt[c][:],
                    start=(c == 0), stop=(c == DC - 1))
            nc.vector.tensor_scalar_add(kt[m][:], pk[:], bk_sb[:, m:m + 1])

        transpose_window(0, hv0, list(range(DC)))

        # weights + inputs needed later, in the order they will be consumed
        wv_sb = []
        wo_sb = []
        for c in range(DC):
            wv = wpool.tile([128, D], BF16, tag="w")
            nc.sync.dma_start(wv[:], wvt_d[c * 128:(c + 1) * 128, :])
            wv_sb.append(wv)
        hv_next = load_hs_window(1)
        for c in range(DC):
            wo = wpool.tile([128, D], BF16, tag="w")
            nc.sync.dma_start(wo[:], wot_d[c * 128:(c + 1) * 128, :])
            wo_sb.append(wo)

        # v_sb[kc][:, h*64:(h+1)*64] = f_all[:, kc*H+h] * (kg_value @ Wv.T)_h
        v_sb = [v_p.tile([128, D], BF16, tag="v", name="vsb")
                for _ in range(KC)]

        def vproj_half(n):
            for kc in range(KC):
                pv = ps.tile([128, 384], F32, tag="mm", bufs=2, name="pv")
                for c in range(DC):
                    nc.tensor.matmul(
                        pv[:], kgt[c][:, kc * 128:(kc + 1) * 128],
                        wv_sb[c][:, n * 384:(n + 1) * 384],
                        start=(c == 0), stop=(c == DC - 1))
                for hh in range(6):
                    h = n * 6 + hh
                    nc.vector.tensor_scalar_mul(
                        v_sb[kc][:, h * 64:(h + 1) * 64],
                        pv[:, hh * 64:(hh + 1) * 64],
                        f_all[:, kc * H + h:kc * H + h + 1])

        def outproj_chunk(tc16):
            # c-outer with n0/n1 interleaved: consecutive matmuls share the
            # same stationary ot[c] tile
            fin = fin_p.tile([128, D], F32, tag="fin")
            pf0 = ps.tile([128, 384], F32, tag="mm", bufs=2, name="pf0")
            pf1 = ps.tile([128, 384], F32, tag="mm", bufs=2, name="pf1")
            for c in range(DC):
                lhs = ot[c][:, tc16 * 128:(tc16 + 1) * 128]
                nc.tensor.matmul(pf0[:], lhs, wo_sb[c][:, 0:384],
                                 start=(c == 0), stop=(c == DC - 1))
                nc.tensor.matmul(pf1[:], lhs, wo_sb[c][:, 384:768],
                                 start=(c == 0), stop=(c == DC - 1))
            nc.vector.tensor_add(fin[:, 0:384], pf0[:], bo_bc[:, 0:384])
            nc.vector.tensor_add(fin[:, 384:768], pf1[:], bo_bc[:, 384:768])
            nc.sync.dma_start(out_d[tc16 * 128:(tc16 + 1) * 128, :], fin[:])

        # denominator weights: dwt[:, (kc*H+h)*64 : +64] = f_all[:, kc*H+h] (x64)
        dwt = const.tile([128, KC * H * 64], BF16, tag="dwt")

        def dwt_build(hs_range):
            for h in hs_range:
                for kc in range(KC):
                    col = kc * H + h
                    nc.vector.tensor_scalar_mul(
                        dwt[:, col * 64:(col + 1) * 64], ones64[:],
                        f_all[:, col:col + 1])

        def dw(kc, h):
            col = kc * H + h
            return dwt[:, col * 64:(col + 1) * 64]

        # ------- phase 2+3: attention + final projection per t-window -------
        ot = [big.tile([128, T], BF16, tag="big", name="ot") for _ in range(NPAIR)]
        for w in range(NTW):
            tw = slice(w * TW, (w + 1) * TW)
            for j in range(NPAIR):
                he, ho = 2 * j, 2 * j + 1
                if w == 0:
                    kproj_chain(j)
                    qproj_chain(0, j)
                # --- scores: even/odd row-tiled into the 2 banks of sc2
                e_tiles = []
                for kc in range(KC):
                    sc2 = ps.tile([128, 2 * TW], F32, tag="s", bufs=2,
                                  name="sc2")
                    nc.tensor.matmul(
                        sc2[:, 0:TW], kt[j][0:64, kc * 128:(kc + 1) * 128],
                        qt[j][0:64, tw], start=True, stop=True,
                        tile_position=(0, 0))
                    nc.tensor.matmul(
                        sc2[:, TW:2 * TW], kt[j][64:128, kc * 128:(kc + 1) * 128],
                        qt[j][64:128, tw], start=True, stop=True,
                        tile_position=(64, 0))
                    ee = e_p.tile([128, 2 * TW], BF16, tag="e", name="ee")
                    if FUSED_EXP:
                        nc.scalar.activation(ee[:], sc2[:], Act.Exp)
                    else:
                        nc.scalar.activation(ee[:, 0:TW], sc2[:, 0:TW], Act.Exp)
                        nc.scalar.activation(
                            ee[:, TW:2 * TW], sc2[:, TW:2 * TW], Act.Exp)
                    e_tiles.append(ee)

                if w == 0 and j == 0:
                    vproj_half(0)
                    dwt_build(range(0, 6))
                elif w == 0 and j == 1:
                    vproj_half(1)
                    dwt_build(range(6, H))

                # --- AV + denominator, col-tiled concurrent chains
                po = ps.tile([128, TW], F32, tag="o", bufs=1, name="po")
                pd = ps.tile([128, TW], F32, tag="d", bufs=1, name="pd")
                # phase A: AV even head (cols 0:64) || denom odd head (cols 64:128)
                for kc in range(KC):
                    nc.tensor.matmul(
                        po[0:64, :], v_sb[kc][:, he * HD:(he + 1) * HD],
                        e_tiles[kc][:, 0:TW],
                        start=(kc == 0), stop=(kc == KC - 1))
                    nc.tensor.matmul(
                        pd[64:128, :], dw(kc, ho),
                        e_tiles[kc][:, TW:2 * TW],
                        start=(kc == 0), stop=(kc == KC - 1))
                # phase B: AV odd head (cols 64:128) || denom even head (cols 0:64)
                for kc in range(KC):
                    nc.tensor.matmul(
                        po[64:128, :], v_sb[kc][:, ho * HD:(ho + 1) * HD],
                        e_tiles[kc][:, TW:2 * TW],
                        start=(kc == 0), stop=(kc == KC - 1))
                    nc.tensor.matmul(
                        pd[0:64, :], dw(kc, he),
                        e_tiles[kc][:, 0:TW],
                        start=(kc == 0), stop=(kc == KC - 1))

                rall = r_p.tile([128, TW], F32, tag="rall", name="rall")
                nc.vector.reciprocal_approx_fast(rall[:], pd[:])
                nc.vector.tensor_mul(ot[j][:, tw], po[:], rall[:])

                # software pipeline: next window's transposes + q-proj,
                # spread across this window's pairs
                if w < NTW - 1:
                    if j == 1:
                        transpose_window(w + 1, hv_next, [0, 1, 2])
                    elif j == 3:
                        transpose_window(w + 1, hv_next, [3, 4, 5])
                    elif j == 4:
                        for m in range(3):
                            qproj_chain(w + 1, m)
                        if w < NTW - 2:
                            hv_next = load_hs_window(w + 2)
                    elif j == 5:
                        for m in range(3, DC):
                            qproj_chain(w + 1, m)

            # --- final projection for this window
            for tsub in range(TW // 128):
                outproj_chunk(w * (TW // 128) + tsub)

    nc.compile()
    return nc


def _get_program():
    if "nc" not in _CACHE:
        _CACHE["nc"] = _build_program()
    return _CACHE["nc"]


def _host_prep(inputs):
    import ml_dtypes
    bf16 = ml_dtypes.bfloat16

    f32 = lambda x: np.ascontiguousarray(np.asarray(x, dtype=np.float32))
    Wq, Wk, Wv, Wo = (f32(inputs[k]) for k in ("Wq", "Wk", "Wv", "Wo"))
    bq, bk, bv, bo = (f32(inputs[k]) for k in ("bq", "bk", "bv", "bo"))
    beta = f32(inputs["beta"])

    shared = {
        "wqt": np.ascontiguousarray((Wq.T * SCALE).astype(bf16)),
        "wkt": np.ascontiguousarray(Wk.T.astype(bf16)),
        "wvt": np.ascontiguousarray(Wv.T.astype(bf16)),
        "wot": np.ascontiguousarray(Wo.T.astype(bf16)),
        "bq": np.ascontiguousarray((bq * SCALE).reshape(DC, 128).T),
        "bk": np.ascontiguousarray(bk.reshape(DC, 128).T),
        # bv folded through Wo (sum_k softmax == 1), bo absorbed:
        "bo": np.ascontiguousarray((bo + bv @ Wo.T).reshape(1, D)),
        "beta": np.ascontiguousarray(beta.reshape(1, H)),
    }

    hs = np.asarray(inputs["hidden_states"])
    kgk = f32(inputs["kg_key"])
    kgv = np.asarray(inputs["kg_value"])
    pooled = f32(inputs["pooled_hidden_states"])
    hs_bf = hs.astype(bf16)
    kgv_bf = kgv.astype(bf16)

    in_maps = []
    for b in range(BS):
        m = dict(shared)
        m["hs"] = np.ascontiguousarray(hs_bf[b])
        m["kgk"] = np.ascontiguousarray(kgk[b])
        m["kgv"] = np.ascontiguousarray(kgv_bf[b])
        m["pooled"] = np.ascontiguousarray(pooled[b].reshape(1, D))
        in_maps.append(m)
    return in_maps




def _install_ntff_hook():
    """Register the axon NTFF profile hook so trace=True yields exec_time_ns.

    Only used from our own test harness (TRACE=True); the default kernel()
    path never calls this.
    """
    try:
        from antenv.axon_hooks import get_axon_ntff_profile_hook  # noqa: F401
        return
    except ImportError:
        pass
    import contextlib
    import ctypes
    import types

    so_path = "/opt/axon/libaxon_pjrt.so"
    try:
        lib = ctypes.CDLL(so_path)
    except OSError:
        return
    if not hasattr(lib, "axon_start_nrt_profile"):
        return
    lib.axon_start_nrt_profile.argtypes = [
        ctypes.POINTER(ctypes.c_int64), ctypes.c_size_t]
    lib.axon_start_nrt_profile.restype = ctypes.c_int64
    lib.axon_stop_nrt_profile.argtypes = [ctypes.c_char_p]
    lib.axon_stop_nrt_profile.restype = ctypes.c_int64

    @contextlib.contextmanager
    def _hook(output_dir, device_ids):
        import jax
        jax.devices()
        if device_ids:
            ids = (ctypes.c_int64 * len(device_ids))(*device_ids)
            rc = lib.axon_start_nrt_profile(ids, len(device_ids))
        else:
            rc = lib.axon_start_nrt_profile(None, 0)
        if rc != 0:
            raise RuntimeError(f"axon_start_nrt_profile rc={rc}")
        try:
            yield
        finally:
            n = lib.axon_stop_nrt_profile(str(output_dir).encode())
            print(f"profile: {n} file(s) written to {output_dir}",
                  file=sys.stderr)

    mod = types.ModuleType("antenv.axon_hooks")
    mod.get_axon_ntff_profile_hook = lambda: _hook
    mod.set_axon_ntff_profile_hook = lambda h: None
    sys.modules["antenv.axon_hooks"] = mod


def kernel(**inputs):
    global LAST_EXEC_NS
    _ensure_path()
    from concourse import bass_utils

    if TRACE:
        _install_ntff_hook()
    nc = _get_program()
    in_maps = _host_prep(inputs)
    res = bass_utils.run_bass_kernel_spmd(
        nc, in_maps, core_ids=list(range(BS)), trace=TRACE)
    LAST_EXEC_NS = res.exec_time_ns
    out = np.stack([res.results[b]["out"] for b in range(BS)], axis=0)
    return out.astype(np.float32)


# revision 27
# speedup vs baseline: 1.0889x; 1.0834x over previous
"""Trainium2 Bass kernel for nn_KnowledgeAttention.

Math (per batch example b):
    sim[k]  = cos_sim(pooled[b], kg_key[b,k])                      # [K]
    q       = (hs @ Wq.T + bq) * HD**-0.5     -> heads [T,H,HD]
    k       = kg_value @ Wk.T + bk            -> heads [K,H,HD]
    v       = kg_value @ Wv.T + bv            -> heads [K,H,HD]
    S[h,t,k]= q_h[t]·k_h[k] + beta[h]*sim[k]
    P       = softmax_k(S);  O[t,h] = sum_k P v
    out     = O @ Wo.T + bo
Since exp(s + beta*sim) = exp(s) * f with f[k,h] = exp(beta[h]*sim[k]),
the per-head bias is folded into f-scaled V columns and f-valued
denominator weights, so the exp itself is bias-free and can process the
even and odd head of a pair in ONE activation instruction over a 2-bank
PSUM tile.

Sharding: pure data-parallel over batch — 8 examples on 8 cores, weights
replicated, no collectives.

Per-core layout (all matmul contractions on the partition dim):
    hs.T/kg_value.T via PE transposes in bf16; scores computed transposed
    S.T[k,t]; even/odd heads of a pair run as row-tiled concurrent
    matmuls into the two halves of a [128,1024] PSUM tile; one bias-free
    Exp covers both; AV and the (64-row-replicated) denominator matmuls
    run col-tiled concurrently; normalization is one reciprocal + one
    multiply per pair.
"""

import sys

import numpy as np

# ---------------------------------------------------------------- constants
BS = 8
T = 2048
D = 768
H = 12
HD = 64
K = 512
SCALE = HD ** -0.5
EPS = 1e-8
DC = D // 128   # 6 contraction/partition chunks of 128 over D
KC = K // 128   # 4 chunks over K
TW = 512        # t window for moving operand
NTW = T // TW   # 4
NPAIR = H // 2  # 6 head pairs

FUSED_EXP = True   # one Exp over [128,1024] spanning 2 PSUM banks

TRACE = False
LAST_EXEC_NS = None

_CACHE = {}


def _ensure_path():
    try:
        import concourse  # noqa: F401
    except ImportError:
        for p in ("/opt/trn_rl_repo", "/root/.axon_site/_ro/trn_rl_repo"):
            if p not in sys.path:
                sys.path.insert(0, p)


def _build_program():
    _ensure_path()
    import concourse.bass as bass
    import concourse.mybir as mybir
    import concourse.tile as tile
    from concourse import bacc
    from concourse.masks import make_identity
    from contextlib import ExitStack

    F32 = mybir.dt.float32
    BF16 = mybir.dt.bfloat16
    Alu = mybir.AluOpType
    Act = mybir.ActivationFunctionType

    nc = bacc.Bacc("TRN2", target_bir_lowering=False, debug=False, num_devices=BS)

    hs_d = nc.dram_tensor("hs", [T, D], BF16, kind="ExternalInput").ap()
    kgk_d = nc.dram_tensor("kgk", [K, D], F32, kind="ExternalInput").ap()
    kgv_d = nc.dram_tensor("kgv", [K, D], BF16, kind="ExternalInput").ap()
    pl_d = nc.dram_tensor("pooled", [1, D], F32, kind="ExternalInput").ap()
    wqt_d = nc.dram_tensor("wqt", [D, D], BF16, kind="ExternalInput").ap()
    wkt_d = nc.dram_tensor("wkt", [D, D], BF16, kind="ExternalInput").ap()
    wvt_d = nc.dram_tensor("wvt", [D, D], BF16, kind="ExternalInput").ap()
    wot_d = nc.dram_tensor("wot", [D, D], BF16, kind="ExternalInput").ap()
    bq_d = nc.dram_tensor("bq", [128, DC], F32, kind="ExternalInput").ap()
    bk_d = nc.dram_tensor("bk", [128, DC], F32, kind="ExternalInput").ap()
    bo_d = nc.dram_tensor("bo", [1, D], F32, kind="ExternalInput").ap()
    beta_d = nc.dram_tensor("beta", [1, H], F32, kind="ExternalInput").ap()
    out_d = nc.dram_tensor("out", [T, D], F32, kind="ExternalOutput").ap()

    with tile.TileContext(nc) as tc, ExitStack() as ctx:
        const = ctx.enter_context(tc.tile_pool(name="const", bufs=1))
        inp = ctx.enter_context(tc.tile_pool(name="inp", bufs=8))
        wpool = ctx.enter_context(tc.tile_pool(name="w", bufs=24))
        big = ctx.enter_context(tc.tile_pool(name="big", bufs=12))
        hstw_p = ctx.enter_context(tc.tile_pool(name="hstw", bufs=12))
        kt_p = ctx.enter_context(tc.tile_pool(name="ktp", bufs=6))
        v_p = ctx.enter_context(tc.tile_pool(name="vp", bufs=4))
        kgt_p = ctx.enter_context(tc.tile_pool(name="kgtp", bufs=6))
        e_p = ctx.enter_context(tc.tile_pool(name="ep", bufs=10))
        r_p = ctx.enter_context(tc.tile_pool(name="rp", bufs=2))
        fin_p = ctx.enter_context(tc.tile_pool(name="finp", bufs=2))
        sm_p = ctx.enter_context(tc.tile_pool(name="smp", bufs=4))
        ps = ctx.enter_context(tc.tile_pool(name="ps", bufs=1, space="PSUM"))

        # ---------------- phase 0: constants + cosine-sim factors ----------------
        # HAM warm-up: keep the PE busy through the whole initial DMA wait so
        # the clock gate opens (K=8/8) and stays open when real work arrives.
        warm = const.tile([128, 512], BF16, tag="warm")
        nc.vector.memset(warm[:], 0.0)
        for i in range(10):
            pwarm = ps.tile([128, 1024], F32, tag="s", bufs=2, name="pwarm")
            nc.tensor.matmul(pwarm[:, 0:512], warm[:, 0:128], warm[:],
                             start=True, stop=True)

        ident = const.tile([128, 128], BF16, tag="ident")
        make_identity(nc, ident[:])
        ones64 = const.tile([128, 64], BF16, tag="ones64")
        nc.vector.memset(ones64[:], 1.0)
        # kg_value + hs window 0 load first: transposes are the critical path
        kv_tiles = []
        for c in range(KC):
            kv = inp.tile([128, D], BF16, tag="inp", name="kv")
            nc.sync.dma_start(kv[:], kgv_d[c * 128:(c + 1) * 128, :])
            kv_tiles.append(kv)

        wk_sb = []
        for c in range(DC):
            wk = wpool.tile([128, D], BF16, tag="w")
            nc.sync.dma_start(wk[:], wkt_d[c * 128:(c + 1) * 128, :])
            wk_sb.append(wk)

        hv0 = []
        for tsub in range(TW // 128):
            hv = inp.tile([128, D], BF16, tag="hv", bufs=10, name="hv")
            nc.sync.dma_start(hv[:], hs_d[tsub * 128:tsub * 128 + 128, :])
            hv0.append(hv)

        wq_sb = []
        for c in range(DC):
            wq = wpool.tile([128, D], BF16, tag="w")
            nc.sync.dma_start(wq[:], wqt_d[c * 128:(c + 1) * 128, :])
            wq_sb.append(wq)

        pl = const.tile([1, D], F32, tag="pl")
        nc.sync.dma_start(pl[:], pl_d)
        bt = const.tile([1, H], F32, tag="bt")
        nc.sync.dma_start(bt[:], beta_d)
        bo_row = const.tile([1, D], F32, tag="bo_row")
        nc.sync.dma_start(bo_row[:], bo_d)
        bq_sb = const.tile([128, DC], F32, tag="bq_sb")
        nc.sync.dma_start(bq_sb[:], bq_d)
        bk_sb = const.tile([128, DC], F32, tag="bk_sb")
        nc.sync.dma_start(bk_sb[:], bk_d)

        bo_bc = const.tile([128, D], F32, tag="bo_bc")
        nc.gpsimd.partition_broadcast(bo_bc[:], bo_row[:], channels=128)
        beta_bc = const.tile([128, H], F32, tag="beta_bc")
        nc.gpsimd.partition_broadcast(beta_bc[:], bt[:], channels=128)
        pl_bc = const.tile([128, D], F32, tag="pl_bc")
        nc.gpsimd.partition_broadcast(pl_bc[:], pl[:], channels=128)

        # pooled 1/||.|| as a per-partition vector (computed on the broadcast)
        pl_sq = inp.tile([128, D], F32, tag="inp")
        pnorm = sm_p.tile([128, 1], F32, tag="pnorm")
        nc.scalar.activation(pl_sq[:], pl_bc[:], Act.Square, accum_out=pnorm[:])
        nc.scalar.activation(pnorm[:], pnorm[:], Act.Sqrt)
        nc.vector.tensor_scalar_max(pnorm[:], pnorm[:], EPS)
        rp_vec = const.tile([128, 1], F32, tag="rp_vec")
        nc.vector.reciprocal(rp_vec[:], pnorm[:])

        # bias_all[k_part, kc*H + h] = beta[h] * sim[k]; f_all = exp(bias_all)
        bias_all = const.tile([128, KC * H], F32, tag="bias_all")
        for c in range(KC):
            kk = inp.tile([128, D], F32, tag="inp")
            nc.sync.dma_start(kk[:], kgk_d[c * 128:(c + 1) * 128, :])
            sq = inp.tile([128, D], F32, tag="inp")
            nrm = sm_p.tile([128, 1], F32, tag="nrm")
            nc.scalar.activation(sq[:], kk[:], Act.Square, accum_out=nrm[:])
            nc.scalar.activation(nrm[:], nrm[:], Act.Sqrt)
            nc.vector.tensor_scalar_max(nrm[:], nrm[:], EPS)
            rn = sm_p.tile([128, 1], F32, tag="rn")
            nc.vector.reciprocal(rn[:], nrm[:])
            sq2 = inp.tile([128, D], F32, tag="inp")
            dot = sm_p.tile([128, 1], F32, tag="dot")
            nc.vector.scalar_tensor_tensor(
                out=sq2[:], in0=kk[:], scalar=1.0, in1=pl_bc[:],
                op0=Alu.mult, op1=Alu.mult, accum_out=dot[:])
            nc.vector.tensor_mul(dot[:], dot[:], rn[:])
            nc.vector.tensor_mul(dot[:], dot[:], rp_vec[:])
            nc.vector.tensor_scalar_mul(
                bias_all[:, c * H:(c + 1) * H], beta_bc[:], dot[:])
        f_all = const.tile([128, KC * H], F32, tag="f_all")
        nc.scalar.activation(f_all[:], bias_all[:], Act.Exp)

        # ---------------- phase 1a: kg_value.T + hs-w0.T ----------------
        kgt = [kgt_p.tile([128, K], BF16, tag="kgt", name="kgt") for _ in range(DC)]
        for dchunk in range(DC):
            pt = ps.tile([128, K], BF16, tag="mm", bufs=2, name="ptr")
            for c in range(KC):
                nc.tensor.transpose(
                    pt[:, c * 128:(c + 1) * 128],
                    kv_tiles[c][:, dchunk * 128:(dchunk + 1) * 128], ident[:])
            nc.vector.tensor_copy(kgt[dchunk][:], pt[:])

        qt = [big.tile([128, T], BF16, tag="big", name="qt") for _ in range(DC)]
        kt = [kt_p.tile([128, K], BF16, tag="kt", name="kt") for _ in range(DC)]
        hstw_w = [[None] * DC for _ in range(NTW)]

        def load_hs_window(w):
            hv_tiles = []
            for tsub in range(TW // 128):
                hv = inp.tile([128, D], BF16, tag="hv", bufs=10, name="hv")
                t0 = w * TW + tsub * 128
                nc.sync.dma_start(hv[:], hs_d[t0:t0 + 128, :])
                hv_tiles.append(hv)
            return hv_tiles

        def transpose_window(w, hv_tiles, cs):
            for c in cs:
                pt = ps.tile([128, TW], BF16, tag="mm", bufs=2, name="ptr")
                for tsub in range(TW // 128):
                    nc.tensor.transpose(
                        pt[:, tsub * 128:(tsub + 1) * 128],
                        hv_tiles[tsub][:, c * 128:(c + 1) * 128], ident[:])
                hstw_w[w][c] = hstw_p.tile(
                    [128, TW], BF16, tag="hstw", name="hstw")
                nc.vector.tensor_copy(hstw_w[w][c][:], pt[:])

        def qproj_chain(w, m):
            pq = ps.tile([128, TW], F32, tag="mm", bufs=2, name="pq")
            for c in range(DC):
                nc.tensor.matmul(
                    pq[:], wq_sb[c][:, m * 128:(m + 1) * 128],
                    hstw_w[w][c][:],
                    start=(c == 0), stop=(c == DC - 1))
            nc.vector.tensor_scalar_add(
                qt[m][:, w * TW:(w + 1) * TW], pq[:], bq_sb[:, m:m + 1])

        def kproj_chain(m):
            pk = ps.tile([128, K], F32, tag="mm", bufs=2, name="pk")
            for c in range(DC):
                nc.tensor.matmul(
                    pk[:], wk_sb[c][:, m * 128:(m + 1) * 128], kgt[c][:],
                    start=(c == 0), stop=(c == DC - 1))
            nc.vector.tensor_scalar_add(kt[m][:], pk[:], bk_sb[:, m:m + 1])

        transpose_window(0, hv0, list(range(DC)))

        # weights + inputs needed later, in the order they will be consumed
        wv_sb = []
        wo_sb = []
        for c in range(DC):
            wv = wpool.tile([128, D], BF16, tag="w")
            nc.sync.dma_start(wv[:], wvt_d[c * 128:(c + 1) * 128, :])
            wv_sb.append(wv)
        hv_next = load_hs_window(1)
        for c in range(DC):
            wo = wpool.tile([128, D], BF16, tag="w")
            nc.sync.dma_start(wo[:], wot_d[c * 128:(c + 1) * 128, :])
            wo_sb.append(wo)

        # v_sb[kc][:, h*64:(h+1)*64] = f_all[:, kc*H+h] * (kg_value @ Wv.T)_h
        v_sb = [v_p.tile([128, D], BF16, tag="v", name="vsb")
                for _ in range(KC)]

        def vproj_half(n):
            for kc in range(KC):
                pv = ps.tile([128, 384], F32, tag="mm", bufs=2, name="pv")
                for c in range(DC):
                    nc.tensor.matmul(
                        pv[:], kgt[c][:, kc * 128:(kc + 1) * 128],
                        wv_sb[c][:, n * 384:(n + 1) * 384],
                        start=(c == 0), stop=(c == DC - 1))
                for hh in range(6):
                    h = n * 6 + hh
                    nc.vector.tensor_scalar_mul(
                        v_sb[kc][:, h * 64:(h + 1) * 64],
                        pv[:, hh * 64:(hh + 1) * 64],
                        f_all[:, kc * H + h:kc * H + h + 1])

        def outproj_chunk(tc16):
            # c-outer with n0/n1 interleaved: consecutive matmuls share the
            # same stationary ot[c] tile
            fin = fin_p.tile([128, D], F32, tag="fin")
            pf0 = ps.tile([128, 384], F32, tag="mm", bufs=2, name="pf0")
            pf1 = ps.tile([128, 384], F32, tag="mm", bufs=2, name="pf1")
            for c in range(DC):
                lhs = ot[c][:, tc16 * 128:(tc16 + 1) * 128]
                nc.tensor.matmul(pf0[:], lhs, wo_sb[c][:, 0:384],
                                 start=(c == 0), stop=(c == DC - 1))
                nc.tensor.matmul(pf1[:], lhs, wo_sb[c][:, 384:768],
                                 start=(c == 0), stop=(c == DC - 1))
            nc.vector.tensor_add(fin[:, 0:384], pf0[:], bo_bc[:, 0:384])
            nc.vector.tensor_add(fin[:, 384:768], pf1[:], bo_bc[:, 384:768])
            nc.sync.dma_start(out_d[tc16 * 128:(tc16 + 1) * 128, :], fin[:])

        # denominator weights: dwt[:, (kc*H+h)*64 : +64] = f_all[:, kc*H+h] (x64)
        dwt = const.tile([128, KC * H * 64], BF16, tag="dwt")

        def dwt_build(hs_range):
            for h in hs_range:
                for kc in range(KC):
                    col = kc * H + h
                    nc.vector.tensor_scalar_mul(
                        dwt[:, col * 64:(col + 1) * 64], ones64[:],
                        f_all[:, col:col + 1])

        def dw(kc, h):
            col = kc * H + h
            return dwt[:, col * 64:(col + 1) * 64]

        # ------- phase 2+3: attention + final projection per t-window -------
        ot = [big.tile([128, T], BF16, tag="big", name="ot") for _ in range(NPAIR)]
        for w in range(NTW):
            tw = slice(w * TW, (w + 1) * TW)
            for j in range(NPAIR):
                he, ho = 2 * j, 2 * j + 1
                if w == 0:
                    kproj_chain(j)
                    qproj_chain(0, j)
                # --- scores: even/odd row-tiled into the 2 banks of sc2
                e_tiles = []
                for kc in range(KC):
                    sc2 = ps.tile([128, 2 * TW], F32, tag="s", bufs=2,
                                  name="sc2")
                    nc.tensor.matmul(
                        sc2[:, 0:TW], kt[j][0:64, kc * 128:(kc + 1) * 128],
                        qt[j][0:64, tw], start=True, stop=True,
                        tile_position=(0, 0))
                    nc.tensor.matmul(
                        sc2[:, TW:2 * TW], kt[j][64:128, kc * 128:(kc + 1) * 128],
                        qt[j][64:128, tw], start=True, stop=True,
                        tile_position=(64, 0))
                    ee = e_p.tile([128, 2 * TW], BF16, tag="e", name="ee")
                    if FUSED_EXP:
                        nc.scalar.activation(ee[:], sc2[:], Act.Exp)
                    else:
                        nc.scalar.activation(ee[:, 0:TW], sc2[:, 0:TW], Act.Exp)
                        nc.scalar.activation(
                            ee[:, TW:2 * TW], sc2[:, TW:2 * TW], Act.Exp)
                    e_tiles.append(ee)

                if w == 0 and j == 0:
                    vproj_half(0)
                    dwt_build(range(0, 6))
                elif w == 0 and j == 1:
                    vproj_half(1)
                    dwt_build(range(6, H))

                # --- AV + denominator, col-tiled concurrent chains
                po = ps.tile([128, TW], F32, tag="o", bufs=1, name="po")
                pd = ps.tile([128, TW], F32, tag="d", bufs=1, name="pd")
                # phase A: AV even head (cols 0:64) || denom odd head (cols 64:128)
                for kc in range(KC):
                    nc.tensor.matmul(
                        po[0:64, :], v_sb[kc][:, he * HD:(he + 1) * HD],
                        e_tiles[kc][:, 0:TW],
                        start=(kc == 0), stop=(kc == KC - 1))
                    nc.tensor.matmul(
                        pd[64:128, :], dw(kc, ho),
                        e_tiles[kc][:, TW:2 * TW],
                        start=(kc == 0), stop=(kc == KC - 1))
                # phase B: AV odd head (cols 64:128) || denom even head (cols 0:64)
                for kc in range(KC):
                    nc.tensor.matmul(
                        po[64:128, :], v_sb[kc][:, ho * HD:(ho + 1) * HD],
                        e_tiles[kc][:, TW:2 * TW],
                        start=(kc == 0), stop=(kc == KC - 1))
                    nc.tensor.matmul(
                        pd[0:64, :], dw(kc, he),
                        e_tiles[kc][:, 0:TW],
                        start=(kc == 0), stop=(kc == KC - 1))

                rall = r_p.tile([128, TW], F32, tag="rall", name="rall")
                nc.vector.reciprocal_approx_fast(rall[:], pd[:])
                nc.vector.tensor_mul(ot[j][:, tw], po[:], rall[:])

                # software pipeline: next window's transposes + q-proj,
                # spread across this window's pairs
                if w < NTW - 1:
                    if j == 1:
                        transpose_window(w + 1, hv_next, [0, 1, 2])
                    elif j == 3:
                        transpose_window(w + 1, hv_next, [3, 4, 5])
                    elif j == 4:
                        for m in range(3):
                            qproj_chain(w + 1, m)
                        if w < NTW - 2:
                            hv_next = load_hs_window(w + 2)
                    elif j == 5:
                        for m in range(3, DC):
                            qproj_chain(w + 1, m)

            # --- final projection for this window
            for tsub in range(TW // 128):
                outproj_chunk(w * (TW // 128) + tsub)

    nc.compile()
    return nc


def _get_program():
    if "nc" not in _CACHE:
        _CACHE["nc"] = _build_program()
    return _CACHE["nc"]


def _host_prep(inputs):
    import ml_dtypes
    bf16 = ml_dtypes.bfloat16

    f32 = lambda x: np.ascontiguousarray(np.asarray(x, dtype=np.float32))
    Wq, Wk, Wv, Wo = (f32(inputs[k]) for k in ("Wq", "Wk", "Wv", "Wo"))
    bq, bk, bv, bo = (f32(inputs[k]) for k in ("bq", "bk", "bv", "bo"))
    beta = f32(inputs["beta"])

    shared = {
        "wqt": np.ascontiguousarray((Wq.T * SCALE).astype(bf16)),
        "wkt": np.ascontiguousarray(Wk.T.astype(bf16)),
        "wvt": np.ascontiguousarray(Wv.T.astype(bf16)),
        "wot": np.ascontiguousarray(Wo.T.astype(bf16)),
        "bq": np.ascontiguousarray((bq * SCALE).reshape(DC, 128).T),
        "bk": np.ascontiguousarray(bk.reshape(DC, 128).T),
        # bv folded through Wo (sum_k softmax == 1), bo absorbed:
        "bo": np.ascontiguousarray((bo + bv @ Wo.T).reshape(1, D)),
        "beta": np.ascontiguousarray(beta.reshape(1, H)),
    }

    hs = np.asarray(inputs["hidden_states"])
    kgk = f32(inputs["kg_key"])
    kgv = np.asarray(inputs["kg_value"])
    pooled = f32(inputs["pooled_hidden_states"])
    hs_bf = hs.astype(bf16)
    kgv_bf = kgv.astype(bf16)

    in_maps = []
    for b in range(BS):
        m = dict(shared)
        m["hs"] = np.ascontiguousarray(hs_bf[b])
        m["kgk"] = np.ascontiguousarray(kgk[b])
        m["kgv"] = np.ascontiguousarray(kgv_bf[b])
        m["pooled"] = np.ascontiguousarray(pooled[b].reshape(1, D))
        in_maps.append(m)
    return in_maps




def _install_ntff_hook():
    """Register the axon NTFF profile hook so trace=True yields exec_time_ns.

    Only used from our own test harness (TRACE=True); the default kernel()
    path never calls this.
    """
    try:
        from antenv.axon_hooks import get_axon_ntff_profile_hook  # noqa: F401
        return
    except ImportError:
        pass
    import contextlib
    import ctypes
    import types

    so_path = "/opt/axon/libaxon_pjrt.so"
    try:
        lib = ctypes.CDLL(so_path)
    except OSError:
        return
    if not hasattr(lib, "axon_start_nrt_profile"):
        return
    lib.axon_start_nrt_profile.argtypes = [
        ctypes.POINTER(ctypes.c_int64), ctypes.c_size_t]
    lib.axon_start_nrt_profile.restype = ctypes.c_int64
    lib.axon_stop_nrt_profile.argtypes = [ctypes.c_char_p]
    lib.axon_stop_nrt_profile.restype = ctypes.c_int64

    @contextlib.contextmanager
    def _hook(output_dir, device_ids):
        import jax
        jax.devices()
        if device_ids:
            ids = (ctypes.c_int64 * len(device_ids))(*device_ids)
            rc = lib.axon_start_nrt_profile(ids, len(device_ids))
        else:
            rc = lib.axon_start_nrt_profile(None, 0)
        if rc != 0:
            raise RuntimeError(f"axon_start_nrt_profile rc={rc}")
        try:
            yield
        finally:
            n = lib.axon_stop_nrt_profile(str(output_dir).encode())
            print(f"profile: {n} file(s) written to {output_dir}",
                  file=sys.stderr)

    mod = types.ModuleType("antenv.axon_hooks")
    mod.get_axon_ntff_profile_hook = lambda: _hook
    mod.set_axon_ntff_profile_hook = lambda h: None
    sys.modules["antenv.axon_hooks"] = mod


def kernel(**inputs):
    global LAST_EXEC_NS
    _ensure_path()
    from concourse import bass_utils

    if TRACE:
        _install_ntff_hook()
    nc = _get_program()
    in_maps = _host_prep(inputs)
    res = bass_utils.run_bass_kernel_spmd(
        nc, in_maps, core_ids=list(range(BS)), trace=TRACE)
    LAST_EXEC_NS = res.exec_time_ns
    out = np.stack([res.results[b]["out"] for b in range(BS)], axis=0)
    return out.astype(np.float32)
